# revision 23
# baseline (speedup 1.0000x reference)
"""Trainium2 Bass kernel for nn_BertDeAttention (dual cross-attention BERT block).

Strategy: data-parallel over batch (8 batches -> 8 NeuronCores). Each core runs
both attention branches for its batch:
  c_out = LN(attn(q=qin, kv=cin; Wq,Wk,Wv) @ Wo.T + bo + cin)
  q_out = LN(attn(q=cin, kv=qin; Wqq,Wqk,Wqv) @ Wo.T + bo + cin)

Device layouts (bf16 matmuls, fp32 accumulation):
  - activations enter feature-major (host-transposed) xT [e, l]
  - Q/K projections produce feature-major [o, l] head-pair tiles
  - V is token-major [l, o] packed per head pair as [v_h0 | ones | v_h1] so a
    single M=128 matmul per head yields both the context rows and the softmax
    denominator (sum of exp) broadcast across 64 partitions
  - scores are computed transposed St[k, q]; the attention mask (per-k) rides
    the ACT per-partition bias slot of the fused exp(0.125*S + m) instruction
  - softmax normalization happens on the PSUM->SBUF pass of the PV output
  - out-projection consumes feature-major ctx tiles; LayerNorm epilogue is
    token-major with bn_stats/bn_aggr
"""
import sys
import numpy as np

sys.path.insert(0, "/opt/trn_rl_repo")

import ml_dtypes  # noqa: E402

VERSION = "v5"
B, L, HID, NH = 8, 1024, 1024, 16
DH = HID // NH  # 64
NP = 128        # partitions
NCH = HID // NP  # 8 chunks of 128 along any 1024 dim
NPAIR = NH // 2  # 8 head pairs
EPS = 1e-12

_COMPILED = {}


def _build(flags):
    import concourse.bass as bass  # noqa: F401
    import concourse.tile as tile
    from concourse import bacc, mybir

    BF16 = mybir.dt.bfloat16
    F32 = mybir.dt.float32
    Alu = mybir.AluOpType
    Act = mybir.ActivationFunctionType

    has_gb = flags["has_gb"]
    reps = flags.get("reps", 1)

    nc = bacc.Bacc("TRN2", target_bir_lowering=False, debug=False)

    # ---- DRAM parameters -------------------------------------------------
    def din(name, shape, dt):
        return nc.dram_tensor(name, shape, dt, kind="ExternalInput").ap()

    xt_c = din("xt_c", [HID, L], BF16)      # cin^T feature-major
    xt_q = din("xt_q", [HID, L], BF16)      # qin^T feature-major
    cin32 = din("cin32", [L, HID], F32)     # residual (token-major)
    mask_c = din("mask_c", [NP, NCH], F32)  # mask[k] at [k%128, k//128]
    mask_q = din("mask_q", [NP, NCH], F32)
    wts = {n: din(f"wt_{n}", [HID, HID], BF16)
           for n in ["q", "k", "v", "qq", "qk", "qv", "o"]}  # W.T ([e, o])
    biases = {n: din(f"b_{n}", [NP, NCH], F32)
              for n in ["q", "k", "qq", "qk"]}               # [o%128, o//128]
    bvbc = {n: din(f"bvbc_{n}", [NP, NCH, NP], F32)
            for n in ["v", "qv"]}                            # bias_v bcast [l, pair, 128]
    if has_gb:
        gb_in = din("gammabeta", [2, HID], F32)

    c_out = nc.dram_tensor("c_out", [L, HID], F32, kind="ExternalOutput").ap()
    q_out = nc.dram_tensor("q_out", [L, HID], F32, kind="ExternalOutput").ap()

    with tile.TileContext(nc) as tc:
        import contextlib
        ctx = contextlib.ExitStack()
        # SBUF pools
        xpool = ctx.enter_context(tc.tile_pool(name="x", bufs=1))
        wtp = ctx.enter_context(tc.tile_pool(name="wt", bufs=6))
        vp = ctx.enter_context(tc.tile_pool(name="vp", bufs=9))
        qkp = ctx.enter_context(tc.tile_pool(name="qkp", bufs=8))
        esp = ctx.enter_context(tc.tile_pool(name="esp", bufs=4))
        ctxp = ctx.enter_context(tc.tile_pool(name="ctxp", bufs=9))
        rbcp = ctx.enter_context(tc.tile_pool(name="rbcp", bufs=2))
        epi = ctx.enter_context(tc.tile_pool(name="epi", bufs=2))
        cinp = ctx.enter_context(tc.tile_pool(name="cinp", bufs=3))
        smal = ctx.enter_context(tc.tile_pool(name="smal", bufs=4))
        # PSUM pools (8 banks total: st 2x2 + pv 2x1 + proj 2x1)
        stp = ctx.enter_context(tc.tile_pool(name="stp", bufs=2, space="PSUM"))
        pvp = ctx.enter_context(tc.tile_pool(name="pvp", bufs=2, space="PSUM"))
        prp = ctx.enter_context(tc.tile_pool(name="prp", bufs=2, space="PSUM"))

        # ---- resident loads ---------------------------------------------
        xc = xpool.tile([NP, NCH, L], BF16, tag="xc")
        nc.sync.dma_start(xc[:], xt_c.rearrange("(c p) l -> p c l", p=NP))
        xq = xpool.tile([NP, NCH, L], BF16, tag="xq")
        nc.sync.dma_start(xq[:], xt_q.rearrange("(c p) l -> p c l", p=NP))

        mset = {}
        for nm, src in [("c", mask_c), ("q", mask_q)]:
            m = smal.tile([NP, NCH], F32, tag=f"mask{nm}")
            nc.sync.dma_start(m[:], src[:])
            mset[nm] = m
        bset = {}
        for nm in ["q", "k", "qq", "qk"]:
            b = smal.tile([NP, NCH], F32, tag=f"b{nm}")
            nc.sync.dma_start(b[:], biases[nm][:])
            bset[nm] = b
        bvset = {}
        for nm in ["v", "qv"]:
            b = xpool.tile([NP, NCH, NP], F32, tag=f"bv{nm}")
            nc.sync.dma_start(b[:], bvbc[nm][:])
            bvset[nm] = b
        eps_sb = smal.tile([NP, 1], F32, tag="eps")
        nc.vector.memset(eps_sb[:], EPS)
        if has_gb:
            gb_bc = xpool.tile([NP, 2, HID], F32, tag="gb")
            import concourse.bass as _b
            gb_src = _b.AP(tensor=gb_in.tensor, offset=gb_in.offset,
                           ap=[[0, NP]] + list(gb_in.ap))
            nc.gpsimd.dma_start(gb_bc[:], gb_src)

        dma_light = flags.get("dma_light", False)

        def load_wt(name):
            """Load W.T as two o-half tiles [128, 8, 512] on the gpsimd
            (SWDGE) ring so weight traffic doesn't queue behind the SP ring;
            halves let downstream matmuls start after 1MB instead of 2MB."""
            halves = []
            src = wts[name].rearrange("(c p) o -> p c o", p=NP)
            for oh in range(2):
                t = wtp.tile([NP, NCH, 512], BF16, tag="wt")
                nc.gpsimd.dma_start(t[:], src[:, :, oh * 512:(oh + 1) * 512])
                halves.append(t)
            return halves

        if dma_light:  # timing-bisect variant: hoist big recurring DMAs
            wshare = load_wt("q")
            cin_share = xpool.tile([NP, HID], F32, tag="cinshare")
            nc.sync.dma_start(cin_share[:], cin32[0:NP, :])
            load_wt = lambda name: wshare  # noqa: E731

        # ================= per-branch program ============================
        loop_cm = tc.For_i(0, reps, 1) if reps > 1 else contextlib.nullcontext()
        ctx.enter_context(loop_cm)
        for br, (xsrc_q, xsrc_kv, wn_q, wn_k, wn_v, msk, out_dram) in {
            "c": (xq, xc, "q", "k", "v", "c", c_out),
            "q": (xc, xq, "qq", "qk", "qv", "q", q_out),
        }.items():
            # ---- V projection: token-major [l, o], packed [v_h0|ones|v_h1]
            wv = load_wt(wn_v)
            bv = bvset[wn_v if wn_v == "v" else "qv"]
            vtiles = []
            for p in range(NPAIR):
                v = vp.tile([NP, NCH, 192], BF16, tag="vp")
                # ones block (cols 64:128 of each k-chunk row)
                nc.vector.memset(v[:, :, 64:128], 1.0)
                vtiles.append(v)
            for oh in range(2):          # o halves of 512
                for lc in range(NCH):    # token chunks
                    ps = prp.tile([NP, 512], F32, tag="pr")
                    for ec in range(NCH):
                        nc.tensor.matmul(
                            ps[:], xsrc_kv[:, ec, lc * NP:(lc + 1) * NP],
                            wv[oh][:, ec, :],
                            start=(ec == 0), stop=(ec == NCH - 1))
                    for pr in range(4):  # pairs within this o-half
                        p = oh * 4 + pr
                        src = ps[:, pr * NP:(pr + 1) * NP].rearrange(
                            "p (h d) -> p h d", h=2)
                        dst = vtiles[p][:, lc, :].rearrange(
                            "p (g d) -> p g d", d=64)  # [p, 3, 64]
                        bsl = bv[:, p, :].rearrange("p (h d) -> p h d", h=2)
                        # dst groups: 0 -> v_h0 (cols 0:64), 2 -> v_h1 (128:192)
                        nc.vector.tensor_tensor(
                            out=_sel2(dst), in0=src, in1=bsl, op=Alu.add)
            # ---- Q/K projections: feature-major pair tiles [128, L]
            qt, kt = [], []
            for wn, bnm, dst_list, xsrc in [
                (wn_q, wn_q, qt, xsrc_q),
                (wn_k, wn_k, kt, xsrc_kv),
            ]:
                w = load_wt(wn)
                bias = bset[bnm]
                for p in range(NPAIR):
                    t = qkp.tile([NP, L], BF16, tag="qt" if dst_list is qt else "kt")
                    for lh in range(2):
                        ps = prp.tile([NP, 512], F32, tag="pr")
                        for ec in range(NCH):
                            nc.tensor.matmul(
                                ps[:], w[p // 4][:, ec, (p % 4) * NP:(p % 4 + 1) * NP],
                                xsrc[:, ec, lh * 512:(lh + 1) * 512],
                                start=(ec == 0), stop=(ec == NCH - 1))
                        nc.vector.tensor_scalar(
                            out=t[:, lh * 512:(lh + 1) * 512], in0=ps[:],
                            scalar1=bias[:, p:p + 1], scalar2=None,
                            op0=Alu.add)
                    dst_list.append(t)
            # ---- attention per head pair --------------------------------
            ctx_tiles = []
            for p in range(NPAIR):
                cx = ctxp.tile([NP, L], BF16, tag="ctx")
                for qh in range(2):
                    qsl = slice(qh * 512, (qh + 1) * 512)
                    pv0 = pvp.tile([NP, 512], F32, tag="pv")
                    pv1 = pvp.tile([NP, 512], F32, tag="pv")
                    for kc in range(NCH):
                        st = stp.tile([NP, 1024], F32, tag="st")
                        # packed score matmuls: h0 cols 0:512, h1 cols 512:1024
                        nc.tensor.matmul(
                            st[:, 0:512],
                            kt[p][0:64, kc * NP:(kc + 1) * NP],
                            qt[p][0:64, qsl],
                            start=True, stop=True)
                        nc.tensor.matmul(
                            st[:, 512:1024],
                            kt[p][64:128, kc * NP:(kc + 1) * NP],
                            qt[p][64:128, qsl],
                            start=True, stop=True)
                        es = esp.tile([NP, 2, 512], BF16, tag="es")
                        nc.scalar.activation(
                            es[:].rearrange("p a b -> p (a b)"), st[:],
                            Act.Exp, bias=mset[msk][:, kc:kc + 1], scale=0.125)
                        nc.tensor.matmul(
                            pv0[:], vtiles[p][:, kc, 0:128], es[:, 0, :],
                            start=(kc == 0), stop=(kc == NCH - 1))
                        nc.tensor.matmul(
                            pv1[:], vtiles[p][:, kc, 64:192], es[:, 1, :],
                            start=(kc == 0), stop=(kc == NCH - 1))
                    # softmax-normalize into feature-major ctx pair tile
                    # pv0: rows 0:64 = ctx_h0, 64:128 = rowsum (bcast)
                    # pv1: rows 0:64 = rowsum (bcast), 64:128 = ctx_h1
                    rbc = rbcp.tile([NP, 512], F32, tag="rbc")
                    nc.vector.reciprocal(rbc[0:64, :], pv0[64:128, :])
                    nc.vector.tensor_tensor(
                        out=cx[0:64, qsl], in0=pv0[0:64, :], in1=rbc[0:64, :],
                        op=Alu.mult)
                    nc.vector.reciprocal(rbc[64:128, :], pv1[0:64, :])
                    nc.vector.tensor_tensor(
                        out=cx[64:128, qsl], in0=pv1[64:128, :],
                        in1=rbc[64:128, :], op=Alu.mult)
                ctx_tiles.append(cx)
            # ---- out projection + residual + LayerNorm ------------------
            wo = load_wt("o")
            for lc in range(NCH):
                pss = []
                for oh in range(2):
                    ps = prp.tile([NP, 512], F32, tag="pr")
                    for ec in range(NCH):
                        nc.tensor.matmul(
                            ps[:], ctx_tiles[ec][:, lc * NP:(lc + 1) * NP],
                            wo[oh][:, ec, :],
                            start=(ec == 0), stop=(ec == NCH - 1))
                    pss.append(ps)
                if dma_light:
                    cint = cin_share
                else:
                    cint = cinp.tile([NP, HID], F32, tag="cin")
                    nc.gpsimd.dma_start(cint[:], cin32[lc * NP:(lc + 1) * NP, :])
                y = epi.tile([NP, HID], F32, tag="y")
                for oh in range(2):
                    nc.vector.tensor_tensor(
                        out=y[:, oh * 512:(oh + 1) * 512], in0=pss[oh][:],
                        in1=cint[:, oh * 512:(oh + 1) * 512], op=Alu.add)
                stats = smal.tile([NP, 2, 6], F32, tag="stats")
                for oh in range(2):
                    nc.vector.bn_stats(stats[:, oh, :], y[:, oh * 512:(oh + 1) * 512])
                mv = smal.tile([NP, 2], F32, tag="mv")
                nc.vector.bn_aggr(mv[:], stats[:])
                # rstd = rsqrt(var + eps) via DVE-only Newton iteration
                # (keeps ACT exp-table resident: Sqrt lives in another table
                # set and would force a reload amid the attention exps).
                # x0 = min(1, 1/v) converges for any v > 0; var here is ~1.
                w = smal.tile([NP, 3], F32, tag="nwt")
                v_ = w[:, 0:1]
                x_ = w[:, 1:2]
                u_ = w[:, 2:3]
                nc.vector.tensor_scalar(out=v_, in0=mv[:, 1:2], scalar1=eps_sb[:],
                                        scalar2=None, op0=Alu.add)
                nc.vector.reciprocal(x_, v_)
                nc.vector.tensor_scalar(out=x_, in0=x_, scalar1=1.0,
                                        scalar2=None, op0=Alu.min)
                for _ in range(3):
                    nc.vector.tensor_tensor(out=u_, in0=x_, in1=x_, op=Alu.mult)
                    nc.vector.tensor_tensor(out=u_, in0=u_, in1=v_, op=Alu.mult)
                    nc.vector.tensor_scalar(out=u_, in0=u_, scalar1=-0.5,
                                            scalar2=1.5, op0=Alu.mult,
                                            op1=Alu.add)
                    nc.vector.tensor_tensor(out=x_, in0=x_, in1=u_, op=Alu.mult)
                o = epi.tile([NP, HID], F32, tag="o")
                nc.vector.tensor_scalar(
                    out=o[:], in0=y[:], scalar1=mv[:, 0:1],
                    scalar2=x_, op0=Alu.subtract, op1=Alu.mult)
                if has_gb:
                    nc.vector.tensor_tensor(
                        out=o[:], in0=o[:], in1=gb_bc[:, 0, :], op=Alu.mult)
                    nc.vector.tensor_tensor(
                        out=o[:], in0=o[:], in1=gb_bc[:, 1, :], op=Alu.add)
                nc.gpsimd.dma_start(out_dram[lc * NP:(lc + 1) * NP, :], o[:])
        ctx.close()
    nc.compile()
    return nc


def _sel2(dst3):
    """dst3 is [p, 3, 64] view ([v_h0 | ones | v_h1]); select groups 0 and 2."""
    import concourse.bass as bass
    ap = dst3.ap  # [[pstep,128],[64,3],[1,64]]
    return bass.AP(tensor=dst3.tensor, offset=dst3.offset,
                   ap=[ap[0], [128, 2], [1, 64]])


def _prep(inputs):
    bf = ml_dtypes.bfloat16

    def t_bf(a):
        return np.ascontiguousarray(np.asarray(a, np.float32).T).astype(bf)

    wts = {}
    for n, key in [("q", "Wq"), ("k", "Wk"), ("v", "Wv"), ("qq", "Wqq"),
                   ("qk", "Wqk"), ("qv", "Wqv"), ("o", "Wo")]:
        wts[n] = t_bf(inputs[key])

    def b_rs(b):
        return np.ascontiguousarray(
            np.asarray(b, np.float32).reshape(NCH, NP).T)

    shared = {f"wt_{n}": w for n, w in wts.items()}
    for n, key in [("q", "bq"), ("k", "bk"), ("qq", "bqq"), ("qk", "bqk")]:
        shared[f"b_{n}"] = b_rs(inputs[key])
    for n, key in [("v", "bv"), ("qv", "bqv")]:
        bb = np.asarray(inputs[key], np.float32)
        shared[f"bvbc_{n}"] = np.ascontiguousarray(
            np.broadcast_to(bb, (NP, HID)).reshape(NP, NCH, NP))
    gamma = np.asarray(inputs["gamma"], np.float32)
    beta = np.asarray(inputs["beta"], np.float32)
    has_gb = not (np.all(gamma == 1.0) and np.all(beta == 0.0))
    if has_gb:
        shared["gammabeta"] = np.ascontiguousarray(
            np.stack([gamma, beta], 0))

    cin = np.asarray(inputs["cinput_tensor"], np.float32)
    qin = np.asarray(inputs["qinput_tensor"], np.float32)
    bo = np.asarray(inputs["bo"], np.float32)  # folded into the residual
    am = np.asarray(inputs["attention_mask"], np.float32).reshape(B, L)
    qam = np.asarray(inputs["qattention_mask"], np.float32).reshape(B, L)

    in_maps = []
    for b in range(B):
        m = dict(shared)
        m["xt_c"] = t_bf(cin[b])
        m["xt_q"] = t_bf(qin[b])
        m["cin32"] = np.ascontiguousarray(cin[b] + bo)
        m["mask_c"] = np.ascontiguousarray(am[b].reshape(NCH, NP).T)
        m["mask_q"] = np.ascontiguousarray(qam[b].reshape(NCH, NP).T)
        in_maps.append(m)
    return in_maps, has_gb


def kernel(**inputs):
    from concourse.bass_utils import run_bass_kernel_spmd

    in_maps, has_gb = _prep(inputs)
    key = (VERSION, has_gb)
    if key not in _COMPILED:
        _COMPILED[key] = _build({"has_gb": has_gb})
    nc = _COMPILED[key]
    res = run_bass_kernel_spmd(nc, in_maps, list(range(B)))
    c = np.stack([res.results[b]["c_out"] for b in range(B)], 0)
    q = np.stack([res.results[b]["q_out"] for b in range(B)], 0)
    return (c, q)
